# revision 1
# baseline (speedup 1.0000x reference)
"""Trainium2 Bass kernel for the CG (Clebsch-Gordan) sparse tensor product.

Math: for each (l, l1, l2) pair, out[b,m,ti,tj] = sum_{m1+m2=m} cg[m1,m2,m] *
(complex product of F_l1[b,m1,ti] and F_l2[b,m2,tj]); outputs concatenated.

Device formulation (per batch-group of G=8 elements, per chunk of pairs that
share the same stationary fragment l1):
  out[(b,ti), (m,tj)] = sum_{(b',m1)} BD[(b',m1),(b,ti)] * RHS[(b',m1),(m,tj)]
where BD is a block-diagonal scatter of F_l1 (so the PE batches G independent
small matmuls in one instruction) and RHS is the cg-scaled, m-shifted copy of
F_l2 built on host. Complex multiply uses the 3-product Gauss trick:
  A = (r1+i1)*T1,  B = r1*(T2-T1),  C = i1*(T1+T2)
  R = A - C, I = A + B          (T1 = cg*r2, T2 = cg*i2)
Each product runs as 3 fp16 matmul passes (hi*hi + hi*lo + lo*hi) where
x = hi + lo is an fp16 two-word split of fp32 (~2^-22 relative error).

The device writes a packed layout ([128 partitions = (b',ti), chunk columns]);
the host reshapes/permutes into the reference output layout at the end.

Sharding: batch 256 -> 32 per core across 8 cores, identical program, per-core
input tables.
"""
import sys
import math

sys.path.insert(0, "/opt/trn_rl_repo")

import numpy as np

# ----------------------------------------------------------------------------
# Problem constants (hardcoded; must match the reference problem definition)
# ----------------------------------------------------------------------------
LMAX = 5
TAU = 16
BATCH = 256
NCORES = 8
BCORE = BATCH // NCORES          # 32 batch elements per core
G = 8                            # batch elements per matmul group (G*16 = 128)
NG = BCORE // G                  # 4 groups per core
CUM = np.concatenate([[0], (TAU * (2 * np.arange(LMAX + 1) + 1)).cumsum()]).astype(int)
MAX_C = 512                      # PSUM bank limit in fp32 columns


def _cg(j1, m1, j2, m2, j, m):
    if m1 + m2 != m or j < abs(j1 - j2) or j > j1 + j2:
        return 0.0
    f = math.factorial
    pref = math.sqrt((2 * j + 1) * f(j + j1 - j2) * f(j - j1 + j2) * f(j1 + j2 - j)
                     / f(j1 + j2 + j + 1))
    pref *= math.sqrt(f(j + m) * f(j - m) * f(j1 - m1) * f(j1 + m1) * f(j2 - m2)
                      * f(j2 + m2))
    kmin = max(0, j2 - j - m1, j1 + m2 - j)
    kmax = min(j1 + j2 - j, j1 - m1, j2 + m2)
    s = 0.0
    for k in range(kmin, kmax + 1):
        s += (-1) ** k / (f(k) * f(j1 + j2 - j - k) * f(j1 - m1 - k) * f(j2 + m2 - k)
                          * f(j - j2 + m1 + k) * f(j - j1 - m2 + k))
    return pref * s


def _pairs():
    out = []
    for l in range(LMAX + 1):
        for l1 in range(LMAX + 1):
            for l2 in range(l1, LMAX + 1):
                if abs(l1 - l2) <= l <= l1 + l2:
                    out.append((l, l1, l2))
    return out


PAIRS = _pairs()
N_L = [sum(1 for (l, _, _) in PAIRS if l == ll) for ll in range(LMAX + 1)]
# idx of each pair within its l-block (reference concatenation order)
IDX_P = {}
for ll in range(LMAX + 1):
    for i, p in enumerate([p for p in PAIRS if p[0] == ll]):
        IDX_P[p] = i
# element offset (dim-1) of each l block in the final output
O_L = np.concatenate([[0], np.cumsum([(2 * l + 1) * 256 * N_L[l]
                                      for l in range(LMAX + 1)])]).astype(int)
OUT_DIM1 = int(O_L[-1])          # 116992


def _cg_mat(l, l1, l2):
    M = np.zeros((2 * l1 + 1, 2 * l2 + 1, 2 * l + 1), dtype=np.float32)
    for m1 in range(-l1, l1 + 1):
        for m2 in range(-l2, l2 + 1):
            m = m1 + m2
            if -l <= m <= l:
                M[m1 + l1, m2 + l2, m + l] = _cg(l1, m1, l2, m2, l, m)
    return M


CG_MATS = {p: _cg_mat(*p) for p in PAIRS}

# ----------------------------------------------------------------------------
# Chunk plan: pairs grouped by stationary fragment l1, packed into <=512-col
# chunks. Each chunk -> one [K=8*(2l1+1), C] rhs block and 9 matmuls.
# ----------------------------------------------------------------------------
CHUNKS = []   # list of dict(l1, K, C, pairs=[(pair, col_off, N_p), ...])
for l1v in range(LMAX + 1):
    cur = None
    for p in PAIRS:
        l, l1, l2 = p
        if l1 != l1v:
            continue
        n_p = (2 * l + 1) * TAU
        if cur is None or cur["C"] + n_p > MAX_C:
            cur = {"l1": l1v, "K": G * (2 * l1v + 1), "C": 0, "pairs": []}
            CHUNKS.append(cur)
        cur["pairs"].append((p, cur["C"], n_p))
        cur["C"] += n_p
# device output column offsets: per group block of 2*sum(C), chunks consecutive
CHUNK_OFF = np.concatenate([[0], np.cumsum([2 * ch["C"] for ch in CHUNKS])]).astype(int)
GCOLS = int(CHUNK_OFF[-1])       # = 2 * 7312 = 14624
DEV_COLS = NG * GCOLS            # 58496

# bd image column layout: (g, l1, var, half) -> 128-wide block
def _bd_col(g, l1, var, half):
    return (((g * (LMAX + 1) + l1) * 3 + var) * 2 + half) * 128

BD_COLS = NG * (LMAX + 1) * 3 * 2 * 128   # 18432

# rhs flat offsets per (g, chunk): block of K * 6C fp16 words
RHS_OFFS = []
_off = 0
for g in range(NG):
    row = []
    for ch in CHUNKS:
        row.append(_off)
        _off += ch["K"] * 6 * ch["C"]
    RHS_OFFS.append(row)
RHS_LEN = _off

# ----------------------------------------------------------------------------
# Host-side packing helpers
# ----------------------------------------------------------------------------
def _split16(a):
    """fp32 -> (hi, lo) fp16 planes with hi+lo ~= a."""
    hi = a.astype(np.float16)
    lo = (a - hi.astype(np.float32)).astype(np.float16)
    return hi, lo


# Per-pair gather/coefficient cache: for rows (m1) and cols (m) of the rhs
# block, the source m2 index (clipped) and the cg coefficient (0 if invalid).
_PAIR_CACHE = {}
def _pair_maps(p):
    if p in _PAIR_CACHE:
        return _PAIR_CACHE[p]
    l, l1, l2 = p
    cg = CG_MATS[p]                                   # (2l1+1, 2l2+1, 2l+1)
    m1g = np.arange(-l1, l1 + 1)[:, None]             # (2l1+1, 1)
    mg = np.arange(-l, l + 1)[None, :]                # (1, 2l+1)
    m2g = mg - m1g                                    # (2l1+1, 2l+1)
    valid = np.abs(m2g) <= l2
    m2c = np.clip(m2g + l2, 0, 2 * l2)
    coeff = cg[np.arange(2 * l1 + 1)[:, None], m2c, np.arange(2 * l + 1)[None, :]]
    coeff = np.where(valid, coeff, 0.0).astype(np.float32)
    _PAIR_CACHE[p] = (m2c, coeff)
    return _PAIR_CACHE[p]


def _build_core_inputs(fs_c):
    """fs_c: (32, 576, 2) fp32 -> dict of per-core device inputs."""
    frR = [fs_c[:, CUM[l]:CUM[l + 1], 0].reshape(BCORE, 2 * l + 1, TAU)
           for l in range(LMAX + 1)]
    frI = [fs_c[:, CUM[l]:CUM[l + 1], 1].reshape(BCORE, 2 * l + 1, TAU)
           for l in range(LMAX + 1)]

    # --- block-diagonal stationary image -------------------------------------
    bd = np.zeros((88, BD_COLS), dtype=np.float16)
    for g in range(NG):
        b0 = g * G
        for l1 in range(LMAX + 1):
            d1 = 2 * l1 + 1
            S = [frR[l1][b0:b0 + G] + frI[l1][b0:b0 + G],     # S1 = r1+i1
                 frR[l1][b0:b0 + G],                          # S2 = r1
                 frI[l1][b0:b0 + G]]                          # S3 = i1
            for var in range(3):
                hi, lo = _split16(S[var].astype(np.float32))
                for half, plane in enumerate((hi, lo)):
                    c0 = _bd_col(g, l1, var, half)
                    for b in range(G):
                        bd[b * d1:(b + 1) * d1, c0 + b * TAU:c0 + (b + 1) * TAU] = \
                            plane[b]

    # --- rhs tables ----------------------------------------------------------
    rhs = np.zeros(RHS_LEN, dtype=np.float16)
    for g in range(NG):
        b0 = g * G
        for ki, ch in enumerate(CHUNKS):
            l1 = ch["l1"]
            K, C = ch["K"], ch["C"]
            d1 = 2 * l1 + 1
            blk = np.zeros((3, K, C), dtype=np.float32)   # U1, U2, U3
            for (p, cp, n_p) in ch["pairs"]:
                l, _, l2 = p
                m2c, coeff = _pair_maps(p)                # (d1, 2l+1)
                r2 = frR[l2][b0:b0 + G]                   # (G, 2l2+1, TAU)
                i2 = frI[l2][b0:b0 + G]
                X = [r2, i2 - r2, r2 + i2]                # T1, T2-T1, T1+T2 bases
                for v in range(3):
                    # gather (G, d1, 2l+1, TAU) then scale by coeff
                    gat = X[v][:, m2c, :]                 # (G, d1, 2l+1, TAU)
                    t = coeff[None, :, :, None] * gat
                    # rows (b', m1) b-major; cols (m, tj)
                    t = t.transpose(0, 1, 2, 3).reshape(G * d1, (2 * l + 1) * TAU)
                    blk[v][:, cp:cp + n_p] = t
            hi1, lo1 = _split16(blk[0])
            hi2, lo2 = _split16(blk[1])
            hi3, lo3 = _split16(blk[2])
            img = np.concatenate([hi1, lo1, hi2, lo2, hi3, lo3], axis=1)  # (K, 6C)
            off = RHS_OFFS[g][ki]
            rhs[off:off + K * 6 * C] = img.reshape(-1)

    return {"bd": bd, "rhs": rhs}


# ----------------------------------------------------------------------------
# Device program (built once, cached)
# ----------------------------------------------------------------------------
_COMPILED = None
def _get_program():
    global _COMPILED
    if _COMPILED is not None:
        return _COMPILED
    import concourse.bass as bass
    import concourse.bacc as bacc
    import concourse.tile as tile
    from concourse import mybir

    f32 = mybir.dt.float32
    f16 = mybir.dt.float16

    nc = bacc.Bacc("TRN2", target_bir_lowering=False, debug=False,
                   num_devices=NCORES)
    bd_d = nc.dram_tensor("bd", (88, BD_COLS), f16, kind="ExternalInput").ap()
    rhs_d = nc.dram_tensor("rhs", (RHS_LEN,), f16, kind="ExternalInput").ap()
    out_d = nc.dram_tensor("out", (128, DEV_COLS), f32, kind="ExternalOutput").ap()

    with tile.TileContext(nc) as tc:
        with tc.tile_pool(name="bdp", bufs=1) as bdp, \
             tc.tile_pool(name="rhsp", bufs=4) as rhsp, \
             tc.tile_pool(name="evp", bufs=3) as evp, \
             tc.tile_pool(name="outp", bufs=3) as outp, \
             tc.tile_pool(name="psp", bufs=2, space=bass.MemorySpace.PSUM) as psp:
            bd_t = bdp.tile([88, BD_COLS], f16)
            nc.sync.dma_start(bd_t[:], bd_d[:])
            for g in range(NG):
                for ki, ch in enumerate(CHUNKS):
                    l1, K, C = ch["l1"], ch["K"], ch["C"]
                    rhs_t = rhsp.tile([K, 6 * C], f16, tag="rhs")
                    off = RHS_OFFS[g][ki]
                    src = rhs_d[off:off + K * 6 * C].rearrange(
                        "(k n) -> k n", n=6 * C)
                    nc.sync.dma_start(rhs_t[:], src)

                    psA = psp.tile([128, C], f32, tag="psA")
                    psB = psp.tile([128, C], f32, tag="psB")
                    psC = psp.tile([128, C], f32, tag="psC")
                    for v, ps in enumerate((psA, psB, psC)):
                        shi = bd_t[0:K, _bd_col(g, l1, v, 0):
                                   _bd_col(g, l1, v, 0) + 128]
                        slo = bd_t[0:K, _bd_col(g, l1, v, 1):
                                   _bd_col(g, l1, v, 1) + 128]
                        uhi = rhs_t[:, (2 * v) * C:(2 * v + 1) * C]
                        ulo = rhs_t[:, (2 * v + 1) * C:(2 * v + 2) * C]
                        nc.tensor.matmul(ps[:], shi, uhi, start=True, stop=False)
                        nc.tensor.matmul(ps[:], shi, ulo, start=False, stop=False)
                        nc.tensor.matmul(ps[:], slo, uhi, start=False, stop=True)

                    a_sb = evp.tile([128, C], f32, tag="asb")
                    nc.scalar.copy(a_sb[:], psA[:])
                    out_t = outp.tile([128, 2 * C], f32, tag="out")
                    nc.vector.tensor_sub(out_t[:, 0:C], a_sb[:], psC[:])
                    nc.vector.tensor_add(out_t[:, C:2 * C], a_sb[:], psB[:])
                    cw = g * GCOLS + int(CHUNK_OFF[ki])
                    nc.sync.dma_start(out_d[:, cw:cw + 2 * C], out_t[:])

    nc.compile()
    _COMPILED = nc
    return nc


# ----------------------------------------------------------------------------
# Entry point
# ----------------------------------------------------------------------------
def kernel(Fs):
    from concourse.bass_utils import run_bass_kernel_spmd

    fs = np.asarray(Fs, dtype=np.float32)
    assert fs.shape == (BATCH, int(CUM[-1]), 2), fs.shape

    nc = _get_program()
    in_maps = [_build_core_inputs(fs[c * BCORE:(c + 1) * BCORE])
               for c in range(NCORES)]
    res = run_bass_kernel_spmd(nc, in_maps, core_ids=list(range(NCORES)))

    out = np.empty((BATCH, OUT_DIM1, 2), dtype=np.float32)
    # per-l views of the final output
    views = [out[:, O_L[l]:O_L[l + 1], :].reshape(
        BATCH, 2 * l + 1, N_L[l], TAU, TAU, 2) for l in range(LMAX + 1)]
    for c in range(NCORES):
        dev = res.results[c]["out"]                    # (128, DEV_COLS)
        dev4 = dev.reshape(128, NG, GCOLS)
        for ki, ch in enumerate(CHUNKS):
            C = ch["C"]
            base = int(CHUNK_OFF[ki])
            for (p, cp, n_p) in ch["pairs"]:
                l = p[0]
                idx = IDX_P[p]
                for half, chn in ((0, 0), (1, 1)):    # R then I
                    cols = base + half * C + cp
                    blk = dev4[:, :, cols:cols + n_p]
                    # (128=(b',ti), NG, (2l+1)*TAU) ->
                    blk = blk.reshape(G, TAU, NG, 2 * l + 1, TAU)
                    blk = blk.transpose(2, 0, 3, 1, 4)  # (g, b', m, ti, tj)
                    views[l][c * BCORE:(c + 1) * BCORE, :, idx, :, :, chn] = \
                        blk.reshape(BCORE, 2 * l + 1, TAU, TAU)
    return out


# revision 2
# speedup vs baseline: 1.8403x; 1.8403x over previous
"""Trainium2 Bass kernel for the CG (Clebsch-Gordan) sparse tensor product.

Math: for each (l, l1, l2) pair, out[b,m,ti,tj] = sum_{m1+m2=m} cg[m1,m2,m] *
(complex product of F_l1[b,m1,ti] and F_l2[b,m2,tj]); outputs concatenated.

Device formulation (per batch-group of G=8 elements, per chunk of pairs that
share the same stationary fragment l1):
  out[(b,ti), (m,tj)] = sum_{(b',m1)} BD[(b',m1),(b,ti)] * RHS[(b',m1),(m,tj)]
where BD is a block-diagonal scatter of F_l1 (the PE batches G independent
small matmuls in one instruction) and RHS is the cg-scaled, m-shifted copy of
F_l2 built on host. Complex multiply uses the 3-product Gauss trick:
  A = (r1+i1)*T1,  B = r1*(T2-T1),  C = i1*(T1+T2)
  R = A - C, I = A + B          (T1 = cg*r2, T2 = cg*i2)

PRECISE=False (default): operands rounded to fp16, one matmul per product
(fp32 PSUM accumulate). Measured end-to-end error ~8e-4 absmax-relative /
resid_var ~2e-7.
PRECISE=True: every operand split hi/lo fp16 (x = hi + lo), three matmul
passes per product -> ~2.6e-7 absmax-relative at ~3x the PE cost.

The device writes a packed layout ([128 partitions = (b',ti), chunk columns]);
the host reshapes/permutes into the reference output layout at the end.

Sharding: batch 256 -> 32 per core across 8 cores, identical program, per-core
input tables.
"""
import sys
import math

sys.path.insert(0, "/opt/trn_rl_repo")

import numpy as np

PRECISE = False

# ----------------------------------------------------------------------------
# Problem constants (hardcoded; must match the reference problem definition)
# ----------------------------------------------------------------------------
LMAX = 5
TAU = 16
BATCH = 256
NCORES = 8
BCORE = BATCH // NCORES          # 32 batch elements per core
G = 8                            # batch elements per matmul group (G*16 = 128)
NG = BCORE // G                  # 4 groups per core
CUM = np.concatenate([[0], (TAU * (2 * np.arange(LMAX + 1) + 1)).cumsum()]).astype(int)
MAX_C = 512                      # PSUM bank limit in fp32 columns
NHALF = 2 if PRECISE else 1      # fp16 words per value
NPLANE = 3 * NHALF               # rhs planes per chunk


def _cg(j1, m1, j2, m2, j, m):
    if m1 + m2 != m or j < abs(j1 - j2) or j > j1 + j2:
        return 0.0
    f = math.factorial
    pref = math.sqrt((2 * j + 1) * f(j + j1 - j2) * f(j - j1 + j2) * f(j1 + j2 - j)
                     / f(j1 + j2 + j + 1))
    pref *= math.sqrt(f(j + m) * f(j - m) * f(j1 - m1) * f(j1 + m1) * f(j2 - m2)
                      * f(j2 + m2))
    kmin = max(0, j2 - j - m1, j1 + m2 - j)
    kmax = min(j1 + j2 - j, j1 - m1, j2 + m2)
    s = 0.0
    for k in range(kmin, kmax + 1):
        s += (-1) ** k / (f(k) * f(j1 + j2 - j - k) * f(j1 - m1 - k) * f(j2 + m2 - k)
                          * f(j - j2 + m1 + k) * f(j - j1 - m2 + k))
    return pref * s


def _pairs():
    out = []
    for l in range(LMAX + 1):
        for l1 in range(LMAX + 1):
            for l2 in range(l1, LMAX + 1):
                if abs(l1 - l2) <= l <= l1 + l2:
                    out.append((l, l1, l2))
    return out


PAIRS = _pairs()
N_L = [sum(1 for (l, _, _) in PAIRS if l == ll) for ll in range(LMAX + 1)]
IDX_P = {}
for ll in range(LMAX + 1):
    for i, p in enumerate([p for p in PAIRS if p[0] == ll]):
        IDX_P[p] = i
O_L = np.concatenate([[0], np.cumsum([(2 * l + 1) * 256 * N_L[l]
                                      for l in range(LMAX + 1)])]).astype(int)
OUT_DIM1 = int(O_L[-1])          # 116992


def _cg_mat(l, l1, l2):
    M = np.zeros((2 * l1 + 1, 2 * l2 + 1, 2 * l + 1), dtype=np.float32)
    for m1 in range(-l1, l1 + 1):
        for m2 in range(-l2, l2 + 1):
            m = m1 + m2
            if -l <= m <= l:
                M[m1 + l1, m2 + l2, m + l] = _cg(l1, m1, l2, m2, l, m)
    return M


CG_MATS = {p: _cg_mat(*p) for p in PAIRS}

# ----------------------------------------------------------------------------
# Chunk plan
# ----------------------------------------------------------------------------
CHUNKS = []   # dict(l1, K, C, pairs=[(pair, col_off, N_p), ...])
for l1v in range(LMAX + 1):
    cur = None
    for p in PAIRS:
        l, l1, l2 = p
        if l1 != l1v:
            continue
        n_p = (2 * l + 1) * TAU
        if cur is None or cur["C"] + n_p > MAX_C:
            cur = {"l1": l1v, "K": G * (2 * l1v + 1), "C": 0, "pairs": []}
            CHUNKS.append(cur)
        cur["pairs"].append((p, cur["C"], n_p))
        cur["C"] += n_p
CHUNK_OFF = np.concatenate([[0], np.cumsum([2 * ch["C"] for ch in CHUNKS])]).astype(int)
GCOLS = int(CHUNK_OFF[-1])       # 14624
DEV_COLS = NG * GCOLS            # 58496


def _bd_col(g, l1, var, half):
    return (((g * (LMAX + 1) + l1) * 3 + var) * NHALF + half) * 128


BD_COLS = NG * (LMAX + 1) * 3 * NHALF * 128

RHS_OFFS = []
_off = 0
for g in range(NG):
    row = []
    for ch in CHUNKS:
        row.append(_off)
        _off += ch["K"] * NPLANE * ch["C"]
    RHS_OFFS.append(row)
RHS_LEN = _off

# ----------------------------------------------------------------------------
# Host-side packing
# ----------------------------------------------------------------------------
def _split16(a):
    hi = a.astype(np.float16)
    if not PRECISE:
        return (hi,)
    lo = (a - hi.astype(np.float32)).astype(np.float16)
    return hi, lo


_PAIR_CACHE = {}
def _pair_maps(p):
    if p in _PAIR_CACHE:
        return _PAIR_CACHE[p]
    l, l1, l2 = p
    cg = CG_MATS[p]
    m1g = np.arange(-l1, l1 + 1)[:, None]
    mg = np.arange(-l, l + 1)[None, :]
    m2g = mg - m1g
    valid = np.abs(m2g) <= l2
    m2c = np.clip(m2g + l2, 0, 2 * l2)
    coeff = cg[np.arange(2 * l1 + 1)[:, None], m2c, np.arange(2 * l + 1)[None, :]]
    coeff = np.where(valid, coeff, 0.0).astype(np.float32)
    _PAIR_CACHE[p] = (m2c, coeff)
    return _PAIR_CACHE[p]


def _build_core_inputs(fs_c):
    frR = [fs_c[:, CUM[l]:CUM[l + 1], 0].reshape(BCORE, 2 * l + 1, TAU)
           for l in range(LMAX + 1)]
    frI = [fs_c[:, CUM[l]:CUM[l + 1], 1].reshape(BCORE, 2 * l + 1, TAU)
           for l in range(LMAX + 1)]

    bd = np.zeros((88, BD_COLS), dtype=np.float16)
    for g in range(NG):
        b0 = g * G
        for l1 in range(LMAX + 1):
            d1 = 2 * l1 + 1
            S = [frR[l1][b0:b0 + G] + frI[l1][b0:b0 + G],
                 frR[l1][b0:b0 + G],
                 frI[l1][b0:b0 + G]]
            for var in range(3):
                for half, plane in enumerate(_split16(S[var].astype(np.float32))):
                    c0 = _bd_col(g, l1, var, half)
                    for b in range(G):
                        bd[b * d1:(b + 1) * d1, c0 + b * TAU:c0 + (b + 1) * TAU] = \
                            plane[b]

    rhs = np.zeros(RHS_LEN, dtype=np.float16)
    for g in range(NG):
        b0 = g * G
        for ki, ch in enumerate(CHUNKS):
            l1 = ch["l1"]
            K, C = ch["K"], ch["C"]
            d1 = 2 * l1 + 1
            blk = np.zeros((3, K, C), dtype=np.float32)
            for (p, cp, n_p) in ch["pairs"]:
                l, _, l2 = p
                m2c, coeff = _pair_maps(p)
                r2 = frR[l2][b0:b0 + G]
                i2 = frI[l2][b0:b0 + G]
                X = [r2, i2 - r2, r2 + i2]
                for v in range(3):
                    gat = X[v][:, m2c, :]
                    t = coeff[None, :, :, None] * gat
                    blk[v][:, cp:cp + n_p] = t.reshape(G * d1, (2 * l + 1) * TAU)
            planes = []
            for v in range(3):
                planes.extend(_split16(blk[v]))
            img = np.concatenate(planes, axis=1)          # (K, NPLANE*C)
            off = RHS_OFFS[g][ki]
            rhs[off:off + K * NPLANE * C] = img.reshape(-1)

    return {"bd": bd, "rhs": rhs}


# ----------------------------------------------------------------------------
# Device program
# ----------------------------------------------------------------------------
_COMPILED = None
def _get_program():
    global _COMPILED
    if _COMPILED is not None:
        return _COMPILED
    import concourse.bass as bass
    import concourse.bacc as bacc
    import concourse.tile as tile
    from concourse import mybir

    f32 = mybir.dt.float32
    f16 = mybir.dt.float16

    nc = bacc.Bacc("TRN2", target_bir_lowering=False, debug=False,
                   num_devices=NCORES)
    bd_d = nc.dram_tensor("bd", (88, BD_COLS), f16, kind="ExternalInput").ap()
    rhs_d = nc.dram_tensor("rhs", (RHS_LEN,), f16, kind="ExternalInput").ap()
    out_d = nc.dram_tensor("out", (128, DEV_COLS), f32, kind="ExternalOutput").ap()

    with tile.TileContext(nc) as tc:
        with tc.tile_pool(name="bdp", bufs=1) as bdp, \
             tc.tile_pool(name="rhsp", bufs=6) as rhsp, \
             tc.tile_pool(name="evp", bufs=3) as evp, \
             tc.tile_pool(name="outp", bufs=3) as outp, \
             tc.tile_pool(name="psp", bufs=2, space=bass.MemorySpace.PSUM) as psp:
            bd_t = bdp.tile([88, BD_COLS], f16)
            nc.sync.dma_start(bd_t[:], bd_d[:])
            for g in range(NG):
                for ki, ch in enumerate(CHUNKS):
                    l1, K, C = ch["l1"], ch["K"], ch["C"]
                    rhs_t = rhsp.tile([K, NPLANE * C], f16, tag="rhs")
                    off = RHS_OFFS[g][ki]
                    src = rhs_d[off:off + K * NPLANE * C].rearrange(
                        "(k n) -> k n", n=NPLANE * C)
                    nc.sync.dma_start(rhs_t[:], src)

                    psA = psp.tile([128, C], f32, tag="psA")
                    psB = psp.tile([128, C], f32, tag="psB")
                    psC = psp.tile([128, C], f32, tag="psC")
                    for v, ps in enumerate((psA, psB, psC)):
                        if PRECISE:
                            shi = bd_t[0:K, _bd_col(g, l1, v, 0):
                                       _bd_col(g, l1, v, 0) + 128]
                            slo = bd_t[0:K, _bd_col(g, l1, v, 1):
                                       _bd_col(g, l1, v, 1) + 128]
                            uhi = rhs_t[:, (2 * v) * C:(2 * v + 1) * C]
                            ulo = rhs_t[:, (2 * v + 1) * C:(2 * v + 2) * C]
                            nc.tensor.matmul(ps[:], shi, uhi, start=True, stop=False)
                            nc.tensor.matmul(ps[:], shi, ulo, start=False, stop=False)
                            nc.tensor.matmul(ps[:], slo, uhi, start=False, stop=True)
                        else:
                            s = bd_t[0:K, _bd_col(g, l1, v, 0):
                                     _bd_col(g, l1, v, 0) + 128]
                            u = rhs_t[:, v * C:(v + 1) * C]
                            nc.tensor.matmul(ps[:], s, u, start=True, stop=True)

                    a_sb = evp.tile([128, C], f32, tag="asb")
                    nc.scalar.copy(a_sb[:], psA[:])
                    out_t = outp.tile([128, 2 * C], f32, tag="out")
                    nc.vector.tensor_sub(out_t[:, 0:C], a_sb[:], psC[:])
                    nc.vector.tensor_add(out_t[:, C:2 * C], a_sb[:], psB[:])
                    cw = g * GCOLS + int(CHUNK_OFF[ki])
                    # output store on the ACT HWDGE ring; inputs on the SP ring
                    nc.scalar.dma_start(out_d[:, cw:cw + 2 * C], out_t[:])

    nc.compile()
    _COMPILED = nc
    return nc


# ----------------------------------------------------------------------------
# Entry point
# ----------------------------------------------------------------------------
def kernel(Fs):
    from concourse.bass_utils import run_bass_kernel_spmd

    fs = np.asarray(Fs, dtype=np.float32)
    assert fs.shape == (BATCH, int(CUM[-1]), 2), fs.shape

    nc = _get_program()
    in_maps = [_build_core_inputs(fs[c * BCORE:(c + 1) * BCORE])
               for c in range(NCORES)]
    res = run_bass_kernel_spmd(nc, in_maps, core_ids=list(range(NCORES)))

    out = np.empty((BATCH, OUT_DIM1, 2), dtype=np.float32)
    views = [out[:, O_L[l]:O_L[l + 1], :].reshape(
        BATCH, 2 * l + 1, N_L[l], TAU, TAU, 2) for l in range(LMAX + 1)]
    for c in range(NCORES):
        dev = res.results[c]["out"]
        dev4 = dev.reshape(128, NG, GCOLS)
        for ki, ch in enumerate(CHUNKS):
            C = ch["C"]
            base = int(CHUNK_OFF[ki])
            for (p, cp, n_p) in ch["pairs"]:
                l = p[0]
                idx = IDX_P[p]
                for half, chn in ((0, 0), (1, 1)):
                    cols = base + half * C + cp
                    blk = dev4[:, :, cols:cols + n_p]
                    blk = blk.reshape(G, TAU, NG, 2 * l + 1, TAU)
                    blk = blk.transpose(2, 0, 3, 1, 4)
                    views[l][c * BCORE:(c + 1) * BCORE, :, idx, :, :, chn] = \
                        blk.reshape(BCORE, 2 * l + 1, TAU, TAU)
    return out


# revision 3
# speedup vs baseline: 2.1547x; 1.1708x over previous
"""Trainium2 Bass kernel for the CG (Clebsch-Gordan) sparse tensor product.

Math: for each (l, l1, l2) pair, out[b,m,ti,tj] = sum_{m1+m2=m} cg[m1,m2,m] *
(complex product of F_l1[b,m1,ti] and F_l2[b,m2,tj]); outputs concatenated.

Device formulation (per batch-group of G=8 elements, per chunk of pairs that
share the same stationary fragment l1):
  out[(b,ti), (m,tj)] = sum_{(b',m1)} BD[(b',m1),(b,ti)] * RHS[(b',m1),(m,tj)]
where BD is a block-diagonal scatter of F_l1 (the PE batches G independent
small matmuls in one instruction) and RHS is the cg-scaled, m-shifted copy of
F_l2 built on host (T1 = cg*r2, T2 = cg*i2). All operands fp16-rounded, fp32
PSUM accumulation (end-to-end error ~8e-4 absmax-relative, resid_var ~2e-7).

Complex multiply, two forms:
 - l1 <= 3 ("stacked"): contraction rows doubled to [T1; T2]; one matmul per
   complex part sharing the same rhs tile:
     R = [r1; -i1]^T [T1; T2],   I = [i1; r1]^T [T1; T2]
 - l1 in {4,5} (2K > 128 rows; "gauss"): 3-product Gauss form
     A = (r1+i1)*T1, B = r1*(T2-T1), C = i1*(T1+T2); R = A-C, I = A+B

The device writes a packed layout ([128 partitions = (b',ti), chunk columns]);
the host reshapes/permutes into the reference output layout at the end.

Sharding: batch 256 -> 32 per core across 8 cores, identical program, per-core
input tables.
"""
import sys
import math

sys.path.insert(0, "/opt/trn_rl_repo")

import numpy as np

# ----------------------------------------------------------------------------
# Problem constants (hardcoded; must match the reference problem definition)
# ----------------------------------------------------------------------------
LMAX = 5
TAU = 16
BATCH = 256
NCORES = 8
BCORE = BATCH // NCORES          # 32 batch elements per core
G = 8                            # batch elements per matmul group (G*16 = 128)
NG = BCORE // G                  # 4 groups per core
CUM = np.concatenate([[0], (TAU * (2 * np.arange(LMAX + 1) + 1)).cumsum()]).astype(int)
MAX_C = 512                      # PSUM bank limit in fp32 columns
FLUSH_COLS = 4096                # batched output tile width (fp32 cols)


def _cg(j1, m1, j2, m2, j, m):
    if m1 + m2 != m or j < abs(j1 - j2) or j > j1 + j2:
        return 0.0
    f = math.factorial
    pref = math.sqrt((2 * j + 1) * f(j + j1 - j2) * f(j - j1 + j2) * f(j1 + j2 - j)
                     / f(j1 + j2 + j + 1))
    pref *= math.sqrt(f(j + m) * f(j - m) * f(j1 - m1) * f(j1 + m1) * f(j2 - m2)
                      * f(j2 + m2))
    kmin = max(0, j2 - j - m1, j1 + m2 - j)
    kmax = min(j1 + j2 - j, j1 - m1, j2 + m2)
    s = 0.0
    for k in range(kmin, kmax + 1):
        s += (-1) ** k / (f(k) * f(j1 + j2 - j - k) * f(j1 - m1 - k) * f(j2 + m2 - k)
                          * f(j - j2 + m1 + k) * f(j - j1 - m2 + k))
    return pref * s


def _pairs():
    out = []
    for l in range(LMAX + 1):
        for l1 in range(LMAX + 1):
            for l2 in range(l1, LMAX + 1):
                if abs(l1 - l2) <= l <= l1 + l2:
                    out.append((l, l1, l2))
    return out


PAIRS = _pairs()
N_L = [sum(1 for (l, _, _) in PAIRS if l == ll) for ll in range(LMAX + 1)]
IDX_P = {}
for ll in range(LMAX + 1):
    for i, p in enumerate([p for p in PAIRS if p[0] == ll]):
        IDX_P[p] = i
O_L = np.concatenate([[0], np.cumsum([(2 * l + 1) * 256 * N_L[l]
                                      for l in range(LMAX + 1)])]).astype(int)
OUT_DIM1 = int(O_L[-1])          # 116992


def _cg_mat(l, l1, l2):
    M = np.zeros((2 * l1 + 1, 2 * l2 + 1, 2 * l + 1), dtype=np.float32)
    for m1 in range(-l1, l1 + 1):
        for m2 in range(-l2, l2 + 1):
            m = m1 + m2
            if -l <= m <= l:
                M[m1 + l1, m2 + l2, m + l] = _cg(l1, m1, l2, m2, l, m)
    return M


CG_MATS = {p: _cg_mat(*p) for p in PAIRS}

# ----------------------------------------------------------------------------
# Chunk plan. mode 's' (stacked, l1<=3): rhs block [2K, C], 2 matmuls.
# mode 'g' (gauss, l1 in {4,5}): rhs block [K, 3C], 3 matmuls + a_sb evict.
# ----------------------------------------------------------------------------
CHUNKS = []
for l1v in range(LMAX + 1):
    mode = "s" if l1v <= 3 else "g"
    cur = None
    for p in PAIRS:
        l, l1, l2 = p
        if l1 != l1v:
            continue
        n_p = (2 * l + 1) * TAU
        if cur is None or cur["C"] + n_p > MAX_C:
            cur = {"l1": l1v, "K": G * (2 * l1v + 1), "C": 0, "pairs": [],
                   "mode": mode}
            CHUNKS.append(cur)
        cur["pairs"].append((p, cur["C"], n_p))
        cur["C"] += n_p
CHUNK_OFF = np.concatenate([[0], np.cumsum([2 * ch["C"] for ch in CHUNKS])]).astype(int)
GCOLS = int(CHUNK_OFF[-1])       # 14624
DEV_COLS = NG * GCOLS            # 58496

# bd image: per (g, l1): 2 blocks (s: W_R, W_I over 2K rows) or 3 blocks
# (g-mode: S1, S2, S3 over K rows); each block 128 cols.
BDCOL = {}
_bc = 0
for g in range(NG):
    for l1 in range(LMAX + 1):
        nblk = 2 if l1 <= 3 else 3
        for i in range(nblk):
            BDCOL[(g, l1, i)] = _bc
            _bc += 128
BD_COLS = _bc                     # 4*(4*2+2*3)*128 = 7168

RHS_OFFS = []
_off = 0
for g in range(NG):
    row = []
    for ch in CHUNKS:
        row.append(_off)
        if ch["mode"] == "s":
            _off += 2 * ch["K"] * ch["C"]
        else:
            _off += ch["K"] * 3 * ch["C"]
    RHS_OFFS.append(row)
RHS_LEN = _off

# output flush groups: consecutive chunks of one g batched into one DMA
FLUSH = []    # (g, first_ki, last_ki, width_cols)
for g in range(NG):
    start, width = 0, 0
    for ki, ch in enumerate(CHUNKS):
        if width + 2 * ch["C"] > FLUSH_COLS and width > 0:
            FLUSH.append((g, start, ki - 1, width))
            start, width = ki, 0
        width += 2 * ch["C"]
    FLUSH.append((g, start, len(CHUNKS) - 1, width))

# ----------------------------------------------------------------------------
# Host-side packing
# ----------------------------------------------------------------------------
_PAIR_CACHE = {}
def _pair_maps(p):
    if p in _PAIR_CACHE:
        return _PAIR_CACHE[p]
    l, l1, l2 = p
    cg = CG_MATS[p]
    m1g = np.arange(-l1, l1 + 1)[:, None]
    mg = np.arange(-l, l + 1)[None, :]
    m2g = mg - m1g
    valid = np.abs(m2g) <= l2
    m2c = np.clip(m2g + l2, 0, 2 * l2)
    coeff = cg[np.arange(2 * l1 + 1)[:, None], m2c, np.arange(2 * l + 1)[None, :]]
    coeff = np.where(valid, coeff, 0.0).astype(np.float32)
    _PAIR_CACHE[p] = (m2c, coeff)
    return _PAIR_CACHE[p]


def _blockdiag(plane):
    """(G, d1, TAU) fp32 -> [G*d1, 128] fp16 block-diagonal."""
    gg, d1, _ = plane.shape
    out = np.zeros((gg * d1, 128), dtype=np.float16)
    for b in range(gg):
        out[b * d1:(b + 1) * d1, b * TAU:(b + 1) * TAU] = plane[b].astype(np.float16)
    return out


def _build_core_inputs(fs_c):
    frR = [fs_c[:, CUM[l]:CUM[l + 1], 0].reshape(BCORE, 2 * l + 1, TAU)
           for l in range(LMAX + 1)]
    frI = [fs_c[:, CUM[l]:CUM[l + 1], 1].reshape(BCORE, 2 * l + 1, TAU)
           for l in range(LMAX + 1)]

    bd = np.zeros((128, BD_COLS), dtype=np.float16)
    for g in range(NG):
        b0 = g * G
        for l1 in range(LMAX + 1):
            d1 = 2 * l1 + 1
            K = G * d1
            r1 = frR[l1][b0:b0 + G]
            i1 = frI[l1][b0:b0 + G]
            if l1 <= 3:
                blocks = [np.concatenate([_blockdiag(r1), _blockdiag(-i1)], axis=0),
                          np.concatenate([_blockdiag(i1), _blockdiag(r1)], axis=0)]
            else:
                blocks = [_blockdiag(r1 + i1), _blockdiag(r1), _blockdiag(i1)]
            for i, blkm in enumerate(blocks):
                c0 = BDCOL[(g, l1, i)]
                bd[0:blkm.shape[0], c0:c0 + 128] = blkm

    rhs = np.zeros(RHS_LEN, dtype=np.float16)
    for g in range(NG):
        b0 = g * G
        for ki, ch in enumerate(CHUNKS):
            l1, K, C = ch["l1"], ch["K"], ch["C"]
            d1 = 2 * l1 + 1
            blk = np.zeros((3, K, C), dtype=np.float32)
            for (p, cp, n_p) in ch["pairs"]:
                l, _, l2 = p
                m2c, coeff = _pair_maps(p)
                r2 = frR[l2][b0:b0 + G]
                i2 = frI[l2][b0:b0 + G]
                if ch["mode"] == "s":
                    X = [r2, i2, None]
                else:
                    X = [r2, i2 - r2, r2 + i2]
                for v in range(2 if ch["mode"] == "s" else 3):
                    gat = X[v][:, m2c, :]
                    t = coeff[None, :, :, None] * gat
                    blk[v][:, cp:cp + n_p] = t.reshape(G * d1, (2 * l + 1) * TAU)
            off = RHS_OFFS[g][ki]
            if ch["mode"] == "s":
                img = np.concatenate([blk[0], blk[1]], axis=0).astype(np.float16)
                rhs[off:off + 2 * K * C] = img.reshape(-1)
            else:
                img = np.concatenate([blk[0], blk[1], blk[2]],
                                     axis=1).astype(np.float16)
                rhs[off:off + K * 3 * C] = img.reshape(-1)

    return {"bd": bd, "rhs": rhs}


# ----------------------------------------------------------------------------
# Device program
# ----------------------------------------------------------------------------
_COMPILED = None
def _get_program():
    global _COMPILED
    if _COMPILED is not None:
        return _COMPILED
    import concourse.bass as bass
    import concourse.bacc as bacc
    import concourse.tile as tile
    from concourse import mybir

    f32 = mybir.dt.float32
    f16 = mybir.dt.float16

    nc = bacc.Bacc("TRN2", target_bir_lowering=False, debug=False,
                   num_devices=NCORES)
    bd_d = nc.dram_tensor("bd", (128, BD_COLS), f16, kind="ExternalInput").ap()
    rhs_d = nc.dram_tensor("rhs", (RHS_LEN,), f16, kind="ExternalInput").ap()
    out_d = nc.dram_tensor("out", (128, DEV_COLS), f32, kind="ExternalOutput").ap()

    flush_of = {}
    for fi, (g, k0, k1, width) in enumerate(FLUSH):
        for ki in range(k0, k1 + 1):
            flush_of[(g, ki)] = fi

    with tile.TileContext(nc) as tc:
        with tc.tile_pool(name="bdp", bufs=1) as bdp, \
             tc.tile_pool(name="rhsp", bufs=6) as rhsp, \
             tc.tile_pool(name="evp", bufs=3) as evp, \
             tc.tile_pool(name="outp", bufs=3) as outp, \
             tc.tile_pool(name="psp", bufs=2, space=bass.MemorySpace.PSUM) as psp:
            bd_t = bdp.tile([128, BD_COLS], f16)
            nc.sync.dma_start(bd_t[:], bd_d[:])
            ev_rr = 0
            for fi, (g, k0, k1, width) in enumerate(FLUSH):
                out_t = outp.tile([128, width], f32, tag="out")
                fbase = int(CHUNK_OFF[k0])
                for ki in range(k0, k1 + 1):
                    ch = CHUNKS[ki]
                    l1, K, C = ch["l1"], ch["K"], ch["C"]
                    off = RHS_OFFS[g][ki]
                    w = int(CHUNK_OFF[ki]) - fbase
                    if ch["mode"] == "s":
                        rhs_t = rhsp.tile([2 * K, C], f16, tag="rhs")
                        src = rhs_d[off:off + 2 * K * C].rearrange(
                            "(k n) -> k n", n=C)
                        nc.sync.dma_start(rhs_t[:], src)
                        psR = psp.tile([128, C], f32, tag="psA")
                        psI = psp.tile([128, C], f32, tag="psB")
                        wR = bd_t[0:2 * K, BDCOL[(g, l1, 0)]:BDCOL[(g, l1, 0)] + 128]
                        wI = bd_t[0:2 * K, BDCOL[(g, l1, 1)]:BDCOL[(g, l1, 1)] + 128]
                        nc.tensor.matmul(psR[:], wR, rhs_t[:], start=True, stop=True)
                        nc.tensor.matmul(psI[:], wI, rhs_t[:], start=True, stop=True)
                        # evict: 2/3 of chunks on DVE, 1/3 on ACT
                        eng = nc.scalar if ev_rr % 3 == 2 else nc.vector
                        if eng is nc.scalar:
                            nc.scalar.copy(out_t[:, w:w + C], psR[:])
                            nc.scalar.copy(out_t[:, w + C:w + 2 * C], psI[:])
                        else:
                            nc.vector.tensor_copy(out_t[:, w:w + C], psR[:])
                            nc.vector.tensor_copy(out_t[:, w + C:w + 2 * C], psI[:])
                        ev_rr += 1
                    else:
                        rhs_t = rhsp.tile([K, 3 * C], f16, tag="rhs")
                        src = rhs_d[off:off + K * 3 * C].rearrange(
                            "(k n) -> k n", n=3 * C)
                        nc.sync.dma_start(rhs_t[:], src)
                        psA = psp.tile([128, C], f32, tag="psA")
                        psB = psp.tile([128, C], f32, tag="psB")
                        psC = psp.tile([128, C], f32, tag="psC")
                        for v, ps in enumerate((psA, psB, psC)):
                            s = bd_t[0:K, BDCOL[(g, l1, v)]:BDCOL[(g, l1, v)] + 128]
                            u = rhs_t[:, v * C:(v + 1) * C]
                            nc.tensor.matmul(ps[:], s, u, start=True, stop=True)
                        a_sb = evp.tile([128, C], f32, tag="asb")
                        nc.scalar.copy(a_sb[:], psA[:])
                        nc.vector.tensor_sub(out_t[:, w:w + C], a_sb[:], psC[:])
                        nc.vector.tensor_add(out_t[:, w + C:w + 2 * C], a_sb[:],
                                             psB[:])
                cw = g * GCOLS + fbase
                eng = nc.scalar if fi % 2 == 0 else nc.sync
                eng.dma_start(out_d[:, cw:cw + width], out_t[:])

    nc.compile()
    _COMPILED = nc
    return nc


# ----------------------------------------------------------------------------
# Entry point
# ----------------------------------------------------------------------------
def kernel(Fs):
    from concourse.bass_utils import run_bass_kernel_spmd

    fs = np.asarray(Fs, dtype=np.float32)
    assert fs.shape == (BATCH, int(CUM[-1]), 2), fs.shape

    nc = _get_program()
    in_maps = [_build_core_inputs(fs[c * BCORE:(c + 1) * BCORE])
               for c in range(NCORES)]
    res = run_bass_kernel_spmd(nc, in_maps, core_ids=list(range(NCORES)))

    out = np.empty((BATCH, OUT_DIM1, 2), dtype=np.float32)
    views = [out[:, O_L[l]:O_L[l + 1], :].reshape(
        BATCH, 2 * l + 1, N_L[l], TAU, TAU, 2) for l in range(LMAX + 1)]
    for c in range(NCORES):
        dev = res.results[c]["out"]
        dev4 = dev.reshape(128, NG, GCOLS)
        for ki, ch in enumerate(CHUNKS):
            C = ch["C"]
            base = int(CHUNK_OFF[ki])
            for (p, cp, n_p) in ch["pairs"]:
                l = p[0]
                idx = IDX_P[p]
                for half, chn in ((0, 0), (1, 1)):
                    cols = base + half * C + cp
                    blk = dev4[:, :, cols:cols + n_p]
                    blk = blk.reshape(G, TAU, NG, 2 * l + 1, TAU)
                    blk = blk.transpose(2, 0, 3, 1, 4)
                    views[l][c * BCORE:(c + 1) * BCORE, :, idx, :, :, chn] = \
                        blk.reshape(BCORE, 2 * l + 1, TAU, TAU)
    return out


# revision 6
# speedup vs baseline: 2.5624x; 1.1892x over previous
"""Trainium2 Bass kernel for the CG (Clebsch-Gordan) sparse tensor product.

Math: for each (l, l1, l2) pair, out[b,m,ti,tj] = sum_{m1+m2=m} cg[m1,m2,m] *
(complex product of F_l1[b,m1,ti] and F_l2[b,m2,tj]); outputs concatenated.

Device formulation (per batch-group of G=8 elements, per chunk of pairs that
share the same stationary fragment l1):
  out[(b,ti), (m,tj)] = sum_{(b',m1)} BD[(b',m1),(b,ti)] * RHS[(b',m1),(m,tj)]
where BD is a block-diagonal scatter of F_l1 (the PE batches G independent
small matmuls in one instruction) and RHS is the cg-scaled, m-shifted copy of
F_l2 built on host (T1 = cg*r2, T2 = cg*i2). All operands fp16-rounded, fp32
PSUM accumulation (end-to-end error ~8e-4 absmax-relative, resid_var ~2e-7).

Complex multiply, two forms:
 - l1 <= 3 ("stacked"): contraction rows doubled to [T1; T2]; one matmul per
   complex part sharing the same rhs tile:
     R = [r1; -i1]^T [T1; T2],   I = [i1; r1]^T [T1; T2]
 - l1 in {4,5} (2K > 128 rows; "gauss"): 3-product Gauss form
     A = (r1+i1)*T1, B = r1*(T2-T1), C = i1*(T1+T2); R = A-C, I = A+B

The device writes a packed layout ([128 partitions = (b',ti), chunk columns]);
the host reshapes/permutes into the reference output layout at the end.

Sharding: batch 256 -> 32 per core across 8 cores, identical program, per-core
input tables.
"""
import sys
import math

sys.path.insert(0, "/opt/trn_rl_repo")

import numpy as np

# ----------------------------------------------------------------------------
# Problem constants (hardcoded; must match the reference problem definition)
# ----------------------------------------------------------------------------
LMAX = 5
TAU = 16
BATCH = 256
NCORES = 8
BCORE = BATCH // NCORES          # 32 batch elements per core
G = 8                            # batch elements per matmul group (G*16 = 128)
NG = BCORE // G                  # 4 groups per core
CUM = np.concatenate([[0], (TAU * (2 * np.arange(LMAX + 1) + 1)).cumsum()]).astype(int)
MAX_C = 512                      # PSUM bank limit in fp32 columns
FLUSH_COLS = 4096                # batched output tile width (fp32 cols)


def _cg(j1, m1, j2, m2, j, m):
    if m1 + m2 != m or j < abs(j1 - j2) or j > j1 + j2:
        return 0.0
    f = math.factorial
    pref = math.sqrt((2 * j + 1) * f(j + j1 - j2) * f(j - j1 + j2) * f(j1 + j2 - j)
                     / f(j1 + j2 + j + 1))
    pref *= math.sqrt(f(j + m) * f(j - m) * f(j1 - m1) * f(j1 + m1) * f(j2 - m2)
                      * f(j2 + m2))
    kmin = max(0, j2 - j - m1, j1 + m2 - j)
    kmax = min(j1 + j2 - j, j1 - m1, j2 + m2)
    s = 0.0
    for k in range(kmin, kmax + 1):
        s += (-1) ** k / (f(k) * f(j1 + j2 - j - k) * f(j1 - m1 - k) * f(j2 + m2 - k)
                          * f(j - j2 + m1 + k) * f(j - j1 - m2 + k))
    return pref * s


def _pairs():
    out = []
    for l in range(LMAX + 1):
        for l1 in range(LMAX + 1):
            for l2 in range(l1, LMAX + 1):
                if abs(l1 - l2) <= l <= l1 + l2:
                    out.append((l, l1, l2))
    return out


PAIRS = _pairs()
N_L = [sum(1 for (l, _, _) in PAIRS if l == ll) for ll in range(LMAX + 1)]
IDX_P = {}
for ll in range(LMAX + 1):
    for i, p in enumerate([p for p in PAIRS if p[0] == ll]):
        IDX_P[p] = i
O_L = np.concatenate([[0], np.cumsum([(2 * l + 1) * 256 * N_L[l]
                                      for l in range(LMAX + 1)])]).astype(int)
OUT_DIM1 = int(O_L[-1])          # 116992


def _cg_mat(l, l1, l2):
    M = np.zeros((2 * l1 + 1, 2 * l2 + 1, 2 * l + 1), dtype=np.float32)
    for m1 in range(-l1, l1 + 1):
        for m2 in range(-l2, l2 + 1):
            m = m1 + m2
            if -l <= m <= l:
                M[m1 + l1, m2 + l2, m + l] = _cg(l1, m1, l2, m2, l, m)
    return M


CG_MATS = {p: _cg_mat(*p) for p in PAIRS}

# ----------------------------------------------------------------------------
# Chunk plan: every chunk "stacked" — rhs rows [T1; T2], one matmul per complex
# part. l1 <= 3: one batch-half (all G=8 elements, 2K rows <= 112). l1 in
# {4,5}: 2K > 128, so the contraction is split into two half-batch matmuls
# (4 elements each, rows 2K/2 <= 88) accumulating into the same PSUM tile.
# ----------------------------------------------------------------------------
CHUNKS = []
for l1v in range(LMAX + 1):
    nh = 1 if l1v <= 3 else 2
    cur = None
    for p in PAIRS:
        l, l1, l2 = p
        if l1 != l1v:
            continue
        n_p = (2 * l + 1) * TAU
        if cur is None or cur["C"] + n_p > MAX_C:
            cur = {"l1": l1v, "K": G * (2 * l1v + 1), "C": 0, "pairs": [],
                   "nh": nh}
            CHUNKS.append(cur)
        cur["pairs"].append((p, cur["C"], n_p))
        cur["C"] += n_p
CHUNK_OFF = np.concatenate([[0], np.cumsum([2 * ch["C"] for ch in CHUNKS])]).astype(int)
GCOLS = int(CHUNK_OFF[-1])       # 14624
DEV_COLS = NG * GCOLS            # 58496

# bd image: per (g, l1, half-group h, part in {R, I}): one 128-col block of
# 2*K/nh rows ([r1; -i1] for R, [i1; r1] for I, block-diagonal per batch elem).
BDCOL = {}
_bc = 0
for g in range(NG):
    for l1 in range(LMAX + 1):
        nh = 1 if l1 <= 3 else 2
        for h in range(nh):
            for i in range(2):
                BDCOL[(g, l1, h, i)] = _bc
                _bc += 128
BD_COLS = _bc

RHS_OFFS = []
_off = 0
for g in range(NG):
    row = []
    for ch in CHUNKS:
        row.append(_off)
        _off += 2 * ch["K"] * ch["C"]
    RHS_OFFS.append(row)
RHS_LEN = _off

# output flush groups: consecutive chunks of one g batched into one DMA
FLUSH = []    # (g, first_ki, last_ki, width_cols)
for g in range(NG):
    start, width = 0, 0
    for ki, ch in enumerate(CHUNKS):
        if width + 2 * ch["C"] > FLUSH_COLS and width > 0:
            FLUSH.append((g, start, ki - 1, width))
            start, width = ki, 0
        width += 2 * ch["C"]
    FLUSH.append((g, start, len(CHUNKS) - 1, width))

# ----------------------------------------------------------------------------
# Host-side packing
# ----------------------------------------------------------------------------
_PAIR_CACHE = {}
def _pair_maps(p):
    if p in _PAIR_CACHE:
        return _PAIR_CACHE[p]
    l, l1, l2 = p
    cg = CG_MATS[p]
    m1g = np.arange(-l1, l1 + 1)[:, None]
    mg = np.arange(-l, l + 1)[None, :]
    m2g = mg - m1g
    valid = np.abs(m2g) <= l2
    m2c = np.clip(m2g + l2, 0, 2 * l2)
    coeff = cg[np.arange(2 * l1 + 1)[:, None], m2c, np.arange(2 * l + 1)[None, :]]
    coeff = np.where(valid, coeff, 0.0).astype(np.float32)
    _PAIR_CACHE[p] = (m2c, coeff)
    return _PAIR_CACHE[p]


def _blockdiag(plane, b_off=0):
    """(n, d1, TAU) fp32 -> [n*d1, 128] fp16 block-diagonal; block b sits at
    columns (b_off+b)*TAU."""
    gg, d1, _ = plane.shape
    out = np.zeros((gg * d1, 128), dtype=np.float16)
    for b in range(gg):
        out[b * d1:(b + 1) * d1, (b_off + b) * TAU:(b_off + b + 1) * TAU] = \
            plane[b].astype(np.float16)
    return out


def _build_core_inputs(fs_c):
    frR = [fs_c[:, CUM[l]:CUM[l + 1], 0].reshape(BCORE, 2 * l + 1, TAU)
           for l in range(LMAX + 1)]
    frI = [fs_c[:, CUM[l]:CUM[l + 1], 1].reshape(BCORE, 2 * l + 1, TAU)
           for l in range(LMAX + 1)]

    bd = np.zeros((128, BD_COLS), dtype=np.float16)
    for g in range(NG):
        b0 = g * G
        for l1 in range(LMAX + 1):
            d1 = 2 * l1 + 1
            K = G * d1
            r1 = frR[l1][b0:b0 + G]
            i1 = frI[l1][b0:b0 + G]
            nh = 1 if l1 <= 3 else 2
            gh = G // nh
            for h in range(nh):
                sl = slice(h * gh, (h + 1) * gh)
                blocks = [np.concatenate([_blockdiag(r1[sl], h * gh),
                                          _blockdiag(-i1[sl], h * gh)], axis=0),
                          np.concatenate([_blockdiag(i1[sl], h * gh),
                                          _blockdiag(r1[sl], h * gh)], axis=0)]
                for i, blkm in enumerate(blocks):
                    c0 = BDCOL[(g, l1, h, i)]
                    bd[0:blkm.shape[0], c0:c0 + 128] = blkm

    rhs = np.zeros(RHS_LEN, dtype=np.float16)
    for g in range(NG):
        b0 = g * G
        for ki, ch in enumerate(CHUNKS):
            l1, K, C = ch["l1"], ch["K"], ch["C"]
            d1 = 2 * l1 + 1
            blk = np.zeros((2, K, C), dtype=np.float32)
            for (p, cp, n_p) in ch["pairs"]:
                l, _, l2 = p
                m2c, coeff = _pair_maps(p)
                r2 = frR[l2][b0:b0 + G]
                i2 = frI[l2][b0:b0 + G]
                for v, X in enumerate((r2, i2)):
                    gat = X[:, m2c, :]
                    t = coeff[None, :, :, None] * gat
                    blk[v][:, cp:cp + n_p] = t.reshape(G * d1, (2 * l + 1) * TAU)
            off = RHS_OFFS[g][ki]
            nh = ch["nh"]
            kh = K // nh
            for h in range(nh):
                img = np.concatenate([blk[0][h * kh:(h + 1) * kh],
                                      blk[1][h * kh:(h + 1) * kh]],
                                     axis=0).astype(np.float16)
                rhs[off + h * 2 * kh * C:off + (h + 1) * 2 * kh * C] = \
                    img.reshape(-1)

    return {"bd": bd, "rhs": rhs}


# ----------------------------------------------------------------------------
# Device program
# ----------------------------------------------------------------------------
_COMPILED = None
def _get_program():
    global _COMPILED
    if _COMPILED is not None:
        return _COMPILED
    import concourse.bass as bass
    import concourse.bacc as bacc
    import concourse.tile as tile
    from concourse import mybir

    f32 = mybir.dt.float32
    f16 = mybir.dt.float16

    nc = bacc.Bacc("TRN2", target_bir_lowering=False, debug=False,
                   num_devices=NCORES)
    bd_d = nc.dram_tensor("bd", (128, BD_COLS), f16, kind="ExternalInput").ap()
    rhs_d = nc.dram_tensor("rhs", (RHS_LEN,), f16, kind="ExternalInput").ap()
    out_d = nc.dram_tensor("out", (128, DEV_COLS), f16, kind="ExternalOutput").ap()

    with tile.TileContext(nc) as tc:
        with tc.tile_pool(name="bdp", bufs=1) as bdp, \
             tc.tile_pool(name="rhsp", bufs=8) as rhsp, \
             tc.tile_pool(name="outp", bufs=3) as outp, \
             tc.tile_pool(name="psp", bufs=2, space=bass.MemorySpace.PSUM) as psp:
            bd_t = bdp.tile([128, BD_COLS], f16)
            nc.sync.dma_start(bd_t[:], bd_d[:])
            ev_rr = 0
            for fi, (g, k0, k1, width) in enumerate(FLUSH):
                out_t = outp.tile([128, width], f16, tag="out")
                fbase = int(CHUNK_OFF[k0])
                for ki in range(k0, k1 + 1):
                    ch = CHUNKS[ki]
                    l1, K, C = ch["l1"], ch["K"], ch["C"]
                    nh = ch["nh"]
                    kh = K // nh
                    off = RHS_OFFS[g][ki]
                    w = int(CHUNK_OFF[ki]) - fbase
                    psR = psp.tile([128, C], f32, tag="psA")
                    psI = psp.tile([128, C], f32, tag="psB")
                    for h in range(nh):
                        rhs_t = rhsp.tile([2 * kh, C], f16, tag="rhs")
                        src_ap = rhs_d[off + h * 2 * kh * C:
                                       off + (h + 1) * 2 * kh * C].rearrange(
                            "(k n) -> k n", n=C)
                        nc.sync.dma_start(rhs_t[:], src_ap)
                        wR = bd_t[0:2 * kh,
                                  BDCOL[(g, l1, h, 0)]:BDCOL[(g, l1, h, 0)] + 128]
                        wI = bd_t[0:2 * kh,
                                  BDCOL[(g, l1, h, 1)]:BDCOL[(g, l1, h, 1)] + 128]
                        nc.tensor.matmul(psR[:], wR, rhs_t[:],
                                         start=(h == 0), stop=(h == nh - 1))
                        nc.tensor.matmul(psI[:], wI, rhs_t[:],
                                         start=(h == 0), stop=(h == nh - 1))
                    # evict with fp16 cast: 2/3 of chunks on DVE, 1/3 on ACT
                    if ev_rr % 3 == 2:
                        nc.scalar.copy(out_t[:, w:w + C], psR[:])
                        nc.scalar.copy(out_t[:, w + C:w + 2 * C], psI[:])
                    else:
                        nc.vector.tensor_copy(out_t[:, w:w + C], psR[:])
                        nc.vector.tensor_copy(out_t[:, w + C:w + 2 * C], psI[:])
                    ev_rr += 1
                cw = g * GCOLS + fbase
                eng = nc.scalar if fi % 2 == 0 else nc.sync
                eng.dma_start(out_d[:, cw:cw + width], out_t[:])

    nc.compile()
    _COMPILED = nc
    return nc


# ----------------------------------------------------------------------------
# Entry point
# ----------------------------------------------------------------------------
def kernel(Fs):
    from concourse.bass_utils import run_bass_kernel_spmd

    fs = np.asarray(Fs, dtype=np.float32)
    assert fs.shape == (BATCH, int(CUM[-1]), 2), fs.shape

    nc = _get_program()
    in_maps = [_build_core_inputs(fs[c * BCORE:(c + 1) * BCORE])
               for c in range(NCORES)]
    res = run_bass_kernel_spmd(nc, in_maps, core_ids=list(range(NCORES)))

    out = np.empty((BATCH, OUT_DIM1, 2), dtype=np.float32)
    views = [out[:, O_L[l]:O_L[l + 1], :].reshape(
        BATCH, 2 * l + 1, N_L[l], TAU, TAU, 2) for l in range(LMAX + 1)]
    for c in range(NCORES):
        dev = res.results[c]["out"]
        dev4 = dev.reshape(128, NG, GCOLS)
        for ki, ch in enumerate(CHUNKS):
            C = ch["C"]
            base = int(CHUNK_OFF[ki])
            for (p, cp, n_p) in ch["pairs"]:
                l = p[0]
                idx = IDX_P[p]
                for half, chn in ((0, 0), (1, 1)):
                    cols = base + half * C + cp
                    blk = dev4[:, :, cols:cols + n_p]
                    blk = blk.reshape(G, TAU, NG, 2 * l + 1, TAU)
                    blk = blk.transpose(2, 0, 3, 1, 4)
                    views[l][c * BCORE:(c + 1) * BCORE, :, idx, :, :, chn] = \
                        blk.reshape(BCORE, 2 * l + 1, TAU, TAU)
    return out
